# revision 4
# baseline (speedup 1.0000x reference)
"""Selective SSM (Mamba-1 style) layer on 8 Trainium2 NeuronCores — v3.

Sharding: core c -> batch b = c // 2, d_model half dh = c % 2 (512 channels).
Cores fully independent (recurrence elementwise in d); no collectives.

The DVE tensor_tensor_scan is the hard bottleneck: 4.42 us per [128,2048]
tile regardless of dtype (2 cyc/elem, no fast modes), 64 tiles = 283 us.
v3 therefore strips the DVE down to scans (+prods) and moves everything
else off it:
  - u = dtx*B_n broadcast muls -> Pool (gpsimd) engine, prefetched ahead.
  - n-reduction: all 8 planes per (half, m) via PE identity-matmul PSUM
    accumulation (PE has slack under the scan envelope).
  - skip term x*D_skip as a 9th PE plane (start plane of half 0).
  - bar exps on ACT (bf16 out, numerically validated).
  - softplus Exp in-place on PSUM (no intermediate tile).
  - y stays bf16; transposed back to [t, d] by SBUF->SBUF DMA xbar,
    stored bf16, upcast in numpy.
"""

import numpy as np
import ml_dtypes
from contextlib import ExitStack

import concourse.bacc as bacc
import concourse.bass as bass
import concourse.mybir as mybir
import concourse.tile as tile
from concourse.bass_utils import run_bass_kernel_spmd

BF16 = ml_dtypes.bfloat16
F32 = mybir.dt.float32
B16 = mybir.dt.bfloat16

B_SZ, SEQ, D, N = 4, 2048, 1024, 16
DL = 512            # d_model channels per core
ND = DL // 128      # 4 d-tiles
NK = D // 128       # 8 contraction tiles
TB = SEQ // 512     # 4 moving-dim blocks for matmul
NHALF = 2           # n-loop halves (SBUF pressure for B/C broadcasts)
NH = N // NHALF     # 8 states per half

_CACHE = {}


def _build():
    if "nc" in _CACHE:
        return _CACHE["nc"]
    mult = mybir.AluOpType.mult
    add = mybir.AluOpType.add

    nc = bacc.Bacc("TRN2", target_bir_lowering=False, debug=False, num_devices=8)

    xb16_d = nc.dram_tensor("xb16", [SEQ, D], B16, kind="ExternalInput")
    xsl16_d = nc.dram_tensor("xsl16", [SEQ, DL], B16, kind="ExternalInput")
    wd16_d = nc.dram_tensor("wd16", [D, DL], B16, kind="ExternalInput")
    wb16_d = nc.dram_tensor("wb16", [D, N], B16, kind="ExternalInput")
    wc16_d = nc.dram_tensor("wc16", [D, N], B16, kind="ExternalInput")
    aneg_d = nc.dram_tensor("aneg", [DL, N], F32, kind="ExternalInput")
    bdsk_d = nc.dram_tensor("bdsk", [DL, 2], F32, kind="ExternalInput")
    bbc_d = nc.dram_tensor("bbc", [N, 2], F32, kind="ExternalInput")
    id16_d = nc.dram_tensor("id16", [128, 128], B16, kind="ExternalInput")
    y_d = nc.dram_tensor("y", [SEQ, DL], B16, kind="ExternalOutput")

    with tile.TileContext(nc) as tc, ExitStack() as ctx:
        consts = ctx.enter_context(tc.tile_pool(name="consts", bufs=1))
        persist = ctx.enter_context(tc.tile_pool(name="persist", bufs=1))
        ps_mm = ctx.enter_context(tc.tile_pool(name="ps_mm", bufs=2, space="PSUM"))
        ps_y = ctx.enter_context(tc.tile_pool(name="ps_y", bufs=1, space="PSUM"))
        big16 = ctx.enter_context(tc.tile_pool(name="big16", bufs=16))
        work = ctx.enter_context(tc.tile_pool(name="work", bufs=2))
        dram = ctx.enter_context(tc.tile_pool(name="dram", bufs=1, space="DRAM"))

        # ---- constants / weights ----
        wdall = consts.tile([128, NK * DL], B16, tag="wdall", name="wdall")
        wball = consts.tile([128, NK * N], B16, tag="wball", name="wball")
        wcall = consts.tile([128, NK * N], B16, tag="wcall", name="wcall")
        for k in range(NK):
            nc.sync.dma_start(wdall[:, k * DL:(k + 1) * DL],
                              wd16_d[k * 128:(k + 1) * 128, :])
            nc.sync.dma_start(wball[:, k * N:(k + 1) * N],
                              wb16_d[k * 128:(k + 1) * 128, :])
            nc.sync.dma_start(wcall[:, k * N:(k + 1) * N],
                              wc16_d[k * 128:(k + 1) * 128, :])
        wd_sb = [wdall[:, k * DL:(k + 1) * DL] for k in range(NK)]
        wb_sb = [wball[:, k * N:(k + 1) * N] for k in range(NK)]
        wc_sb = [wcall[:, k * N:(k + 1) * N] for k in range(NK)]
        abd = []
        for m in range(ND):
            t = consts.tile([128, N + 2], F32, tag=f"abd{m}", name=f"abd{m}")
            nc.sync.dma_start(t[:, 0:N], aneg_d[m * 128:(m + 1) * 128, :])
            nc.sync.dma_start(t[:, N:N + 2], bdsk_d[m * 128:(m + 1) * 128, :])
            abd.append(t)
        a_sb = [t[:, 0:N] for t in abd]
        bd_sb = [t[:, N:N + 1] for t in abd]
        dsk_sb = [t[:, N + 1:N + 2] for t in abd]
        bbc = consts.tile([N, 2], F32, tag="bbc", name="bbc")
        nc.sync.dma_start(bbc[:], bbc_d[:, :])
        bb_sb = bbc[:, 0:1]
        bc_sb = bbc[:, 1:2]
        id16_sb = consts.tile([128, 128], B16, tag="id16", name="id16sb")
        nc.sync.dma_start(id16_sb[:], id16_d[:, :])

        # ---- x^T via DMA xbar transpose ----
        xt = []
        for k in range(NK):
            t = big16.tile([128, SEQ], B16, tag="big16", name=f"xt{k}")
            nc.sync.dma_start_transpose(t[:], xb16_d[:, k * 128:(k + 1) * 128])
            xt.append(t)
        xts = []
        for m in range(ND):
            t = big16.tile([128, SEQ], B16, tag="big16", name=f"xts{m}")
            nc.sync.dma_start_transpose(t[:], xsl16_d[:, m * 128:(m + 1) * 128])
            xts.append(t)

        # ---- B/C projections -> bmat/cmat [N, SEQ] bf16 ----
        bmat = persist.tile([N, SEQ], B16, tag="bmat", name="bmat")
        cmat = persist.tile([N, SEQ], B16, tag="cmat", name="cmat")
        for tb in range(TB):
            psb = ps_mm.tile([128, 1024], F32, tag="mm", name="mmpb")
            for k in range(NK):
                nc.tensor.matmul(
                    psb[0:N, 0:512], wb_sb[k], xt[k][:, tb * 512:(tb + 1) * 512],
                    start=(k == 0), stop=(k == NK - 1),
                )
            for k in range(NK):
                nc.tensor.matmul(
                    psb[0:N, 512:1024], wc_sb[k],
                    xt[k][:, tb * 512:(tb + 1) * 512],
                    start=(k == 0), stop=(k == NK - 1),
                )
            nc.scalar.activation(
                bmat[:, tb * 512:(tb + 1) * 512], psb[0:N, 0:512],
                mybir.ActivationFunctionType.Identity, bias=bb_sb, scale=1.0,
            )
            nc.scalar.activation(
                cmat[:, tb * 512:(tb + 1) * 512], psb[0:N, 512:1024],
                mybir.ActivationFunctionType.Identity, bias=bc_sb, scale=1.0,
            )

        # bounce B/C to DRAM for partition-broadcast reads
        bmat_dr = dram.tile([N, SEQ], B16, tag="bmat_dr", name="bmat_dr")
        cmat_dr = dram.tile([N, SEQ], B16, tag="cmat_dr", name="cmat_dr")
        nc.sync.dma_start(bmat_dr[:], bmat[:])
        nc.sync.dma_start(cmat_dr[:], cmat[:])

        # ---- delta projection + softplus (Exp in-place on PSUM) ----
        dt16 = [persist.tile([128, SEQ], B16, tag=f"dt{m}", name=f"dtv{m}")
                for m in range(ND)]
        for m in range(ND):
            pss = []
            for th in range(2):
                ps = ps_mm.tile([128, 1024], F32, tag="mm", name="mmps")
                for sb in range(2):
                    for k in range(NK):
                        nc.tensor.matmul(
                            ps[:, sb * 512:(sb + 1) * 512],
                            wd_sb[k][:, m * 128:(m + 1) * 128],
                            xt[k][:, th * 1024 + sb * 512:th * 1024 + (sb + 1) * 512],
                            start=(k == 0), stop=(k == NK - 1),
                        )
                nc.scalar.activation(
                    ps[:], ps[:], mybir.ActivationFunctionType.Exp,
                    bias=bd_sb[m], scale=1.0,
                )
                pss.append(ps)
            for th in range(2):
                nc.scalar.activation(
                    dt16[m][:, th * 1024:(th + 1) * 1024], pss[th][:],
                    mybir.ActivationFunctionType.Ln, bias=1.0, scale=1.0,
                )

        # ---- dtx[m] = dt16[m]*x^T_own;  dskx[m] = x^T_own*D_skip (DVE) ----
        dtx = [persist.tile([128, SEQ], B16, tag=f"dtx{m}", name=f"dtx{m}")
               for m in range(ND)]
        dskx = [persist.tile([128, SEQ], B16, tag=f"dskx{m}", name=f"dskx{m}")
                for m in range(ND)]
        for m in range(ND):
            nc.vector.tensor_mul(dtx[m][:], dt16[m][:], xts[m][:])
            nc.vector.tensor_scalar_mul(dskx[m][:], xts[m][:], dsk_sb[m])

        # ---- scan phase ----
        y16 = [persist.tile([128, SEQ], B16, tag=f"y16{m}", name=f"y16{m}")
               for m in range(ND)]
        for half in range(NHALF):
            breps = []
            creps = []
            for j in range(NH):
                n = half * NH + j
                br = big16.tile([128, SEQ], B16, tag="big16", name="brep")
                nc.sync.dma_start(br[:], bmat_dr[n:n + 1, :].partition_broadcast(128))
                breps.append(br)
                cr = big16.tile([128, SEQ], B16, tag="big16", name="crep")
                nc.sync.dma_start(cr[:], cmat_dr[n:n + 1, :].partition_broadcast(128))
                creps.append(cr)
            for m in range(ND):
                yps = ps_y.tile([128, SEQ], F32, tag="yps", name="ypsv")
                # skip plane opens the accumulation group in half 0
                if half == 0:
                    for tb in range(TB):
                        nc.tensor.matmul(
                            yps[:, tb * 512:(tb + 1) * 512], id16_sb[:],
                            dskx[m][:, tb * 512:(tb + 1) * 512],
                            start=True, stop=False,
                        )
                for j in range(NH):
                    n = half * NH + j
                    bar = work.tile([128, SEQ], B16, tag="bar", name="barv",
                                    bufs=3)
                    nc.scalar.activation(
                        bar[:], dt16[m][:],
                        mybir.ActivationFunctionType.Exp,
                        bias=0.0, scale=a_sb[m][:, n:n + 1],
                    )
                    u = work.tile([128, SEQ], B16, tag="u", name="uv", bufs=4)
                    nc.gpsimd.tensor_mul(u[:], dtx[m][:], breps[j][:])
                    h = work.tile([128, SEQ], B16, tag="h", name="hv")
                    nc.vector.tensor_tensor_scan(
                        h[:], bar[:], u[:], 0.0, op0=mult, op1=add,
                    )
                    prod = work.tile([128, SEQ], B16, tag="prod", name="prodv",
                                     bufs=4)
                    nc.vector.tensor_mul(prod[:], h[:], creps[j][:])
                    first = (half == 1 and j == 0)
                    last = (j == NH - 1)
                    for tb in range(TB):
                        nc.tensor.matmul(
                            yps[:, tb * 512:(tb + 1) * 512], id16_sb[:],
                            prod[:, tb * 512:(tb + 1) * 512],
                            start=first, stop=last,
                        )
                if half == 0:
                    nc.scalar.activation(
                        y16[m][:], yps[:],
                        mybir.ActivationFunctionType.Copy, bias=0.0, scale=1.0,
                    )
                else:
                    nc.vector.tensor_add(y16[m][:], y16[m][:], yps[:])

        # ---- y^T via SBUF->SBUF xbar transpose, store bf16 ----
        for m in range(ND):
            ytt = work.tile([128, SEQ // 128, 128], B16, tag="ytt", name="yttv")
            nc.sync.dma_start_transpose(ytt[:], y16[m][:])
            nc.sync.dma_start(
                y_d[:, m * 128:(m + 1) * 128]
                .rearrange("(j p) q -> p j q", p=128),
                ytt[:],
            )

    nc.compile()
    _CACHE["nc"] = nc
    return nc


def _in_maps(x, A_log, D_skip, Wd, bd, Wb, bb, Wc, bc):
    A = (-np.exp(np.asarray(A_log, np.float64))).astype(np.float32)
    x = np.asarray(x, np.float32)
    maps = []
    for c in range(8):
        b, dh = c // 2, c % 2
        dsl = slice(dh * DL, (dh + 1) * DL)
        bdsk = np.stack([np.asarray(bd, np.float32)[dsl],
                         np.asarray(D_skip, np.float32)[dsl]], axis=1)
        bbcm = np.stack([np.asarray(bb, np.float32),
                         np.asarray(bc, np.float32)], axis=1)
        maps.append({
            "xb16": x[b].astype(BF16),
            "xsl16": x[b][:, dsl].astype(BF16),
            "wd16": np.asarray(Wd)[:, dsl].astype(BF16),
            "wb16": np.asarray(Wb).astype(BF16),
            "wc16": np.asarray(Wc).astype(BF16),
            "aneg": A[dsl],
            "bdsk": np.ascontiguousarray(bdsk),
            "bbc": np.ascontiguousarray(bbcm),
            "id16": np.eye(128, dtype=BF16),
        })
    return maps


def kernel(x, A_log, D_skip, Wd, bd, Wb, bb, Wc, bc, _trace=False):
    nc = _build()
    maps = _in_maps(x, A_log, D_skip, Wd, bd, Wb, bb, Wc, bc)
    res = run_bass_kernel_spmd(nc, maps, list(range(8)), trace=_trace)
    y = np.zeros((B_SZ, SEQ, D), np.float32)
    for c, om in enumerate(res.results):
        b, dh = c // 2, c % 2
        y[b][:, dh * DL:(dh + 1) * DL] = om["y"].astype(np.float32)
    if _trace:
        kernel.last_result = res
    return y


# revision 5
# speedup vs baseline: 1.5622x; 1.5622x over previous
"""Selective SSM (Mamba-1 style) layer on 8 Trainium2 NeuronCores — v3.

Sharding: core c -> batch b = c // 2, d_model half dh = c % 2 (512 channels).
Cores fully independent (recurrence elementwise in d); no collectives.

The DVE tensor_tensor_scan is the hard bottleneck: 4.42 us per [128,2048]
tile regardless of dtype (2 cyc/elem, no fast modes), 64 tiles = 283 us.
Concurrent Pool-engine tensor ops poison SBUF bandwidth (v3 measured DVE
muls 1.07 -> 5.0 us), so the Pool stays idle; DVE does scans + u/prod muls
(2x mode, 1.07 us each) and everything else hides under it:
  - n-reduction: 8 planes per (half, m) via PE identity-matmul PSUM
    accumulation; skip term x*D_skip opens half 0, the running y16 opens
    half 1 (so no DVE merge adds); ACT copies PSUM -> y16 bf16.
  - bar exps on ACT (bf16 out, numerically validated).
  - softplus Exp in-place on PSUM (no intermediate tile).
  - y stays bf16; transposed back to [t, d] by SBUF->SBUF DMA xbar,
    stored bf16, upcast in numpy.
"""

import numpy as np
import ml_dtypes
from contextlib import ExitStack

import concourse.bacc as bacc
import concourse.bass as bass
import concourse.mybir as mybir
import concourse.tile as tile
from concourse.bass_utils import run_bass_kernel_spmd

BF16 = ml_dtypes.bfloat16
F32 = mybir.dt.float32
B16 = mybir.dt.bfloat16

B_SZ, SEQ, D, N = 4, 2048, 1024, 16
DL = 512            # d_model channels per core
ND = DL // 128      # 4 d-tiles
NK = D // 128       # 8 contraction tiles
TB = SEQ // 512     # 4 moving-dim blocks for matmul
NHALF = 2           # n-loop halves (SBUF pressure for B/C broadcasts)
NH = N // NHALF     # 8 states per half

_CACHE = {}


def _build():
    if "nc" in _CACHE:
        return _CACHE["nc"]
    mult = mybir.AluOpType.mult
    add = mybir.AluOpType.add

    nc = bacc.Bacc("TRN2", target_bir_lowering=False, debug=False, num_devices=8)

    xb16_d = nc.dram_tensor("xb16", [SEQ, D], B16, kind="ExternalInput")
    xsl16_d = nc.dram_tensor("xsl16", [SEQ, DL], B16, kind="ExternalInput")
    wd16_d = nc.dram_tensor("wd16", [D, DL], B16, kind="ExternalInput")
    wb16_d = nc.dram_tensor("wb16", [D, N], B16, kind="ExternalInput")
    wc16_d = nc.dram_tensor("wc16", [D, N], B16, kind="ExternalInput")
    aneg_d = nc.dram_tensor("aneg", [DL, N], F32, kind="ExternalInput")
    bdsk_d = nc.dram_tensor("bdsk", [DL, 2], F32, kind="ExternalInput")
    bbc_d = nc.dram_tensor("bbc", [N, 2], F32, kind="ExternalInput")
    id16_d = nc.dram_tensor("id16", [128, 128], B16, kind="ExternalInput")
    y_d = nc.dram_tensor("y", [SEQ, DL], B16, kind="ExternalOutput")

    with tile.TileContext(nc) as tc, ExitStack() as ctx:
        consts = ctx.enter_context(tc.tile_pool(name="consts", bufs=1))
        persist = ctx.enter_context(tc.tile_pool(name="persist", bufs=1))
        ps_mm = ctx.enter_context(tc.tile_pool(name="ps_mm", bufs=2, space="PSUM"))
        ps_y = ctx.enter_context(tc.tile_pool(name="ps_y", bufs=1, space="PSUM"))
        big16 = ctx.enter_context(tc.tile_pool(name="big16", bufs=16))
        work = ctx.enter_context(tc.tile_pool(name="work", bufs=2))
        dram = ctx.enter_context(tc.tile_pool(name="dram", bufs=1, space="DRAM"))

        # ---- constants / weights ----
        wdall = consts.tile([128, NK * DL], B16, tag="wdall", name="wdall")
        wball = consts.tile([128, NK * N], B16, tag="wball", name="wball")
        wcall = consts.tile([128, NK * N], B16, tag="wcall", name="wcall")
        for k in range(NK):
            nc.sync.dma_start(wdall[:, k * DL:(k + 1) * DL],
                              wd16_d[k * 128:(k + 1) * 128, :])
            nc.sync.dma_start(wball[:, k * N:(k + 1) * N],
                              wb16_d[k * 128:(k + 1) * 128, :])
            nc.sync.dma_start(wcall[:, k * N:(k + 1) * N],
                              wc16_d[k * 128:(k + 1) * 128, :])
        wd_sb = [wdall[:, k * DL:(k + 1) * DL] for k in range(NK)]
        wb_sb = [wball[:, k * N:(k + 1) * N] for k in range(NK)]
        wc_sb = [wcall[:, k * N:(k + 1) * N] for k in range(NK)]
        abd = []
        for m in range(ND):
            t = consts.tile([128, N + 2], F32, tag=f"abd{m}", name=f"abd{m}")
            nc.sync.dma_start(t[:, 0:N], aneg_d[m * 128:(m + 1) * 128, :])
            nc.sync.dma_start(t[:, N:N + 2], bdsk_d[m * 128:(m + 1) * 128, :])
            abd.append(t)
        a_sb = [t[:, 0:N] for t in abd]
        bd_sb = [t[:, N:N + 1] for t in abd]
        dsk_sb = [t[:, N + 1:N + 2] for t in abd]
        bbc = consts.tile([N, 2], F32, tag="bbc", name="bbc")
        nc.sync.dma_start(bbc[:], bbc_d[:, :])
        bb_sb = bbc[:, 0:1]
        bc_sb = bbc[:, 1:2]
        id16_sb = consts.tile([128, 128], B16, tag="id16", name="id16sb")
        nc.sync.dma_start(id16_sb[:], id16_d[:, :])

        # ---- x^T via DMA xbar transpose ----
        xt = []
        for k in range(NK):
            t = big16.tile([128, SEQ], B16, tag="big16", name=f"xt{k}")
            nc.sync.dma_start_transpose(t[:], xb16_d[:, k * 128:(k + 1) * 128])
            xt.append(t)
        xts = []
        for m in range(ND):
            t = big16.tile([128, SEQ], B16, tag="big16", name=f"xts{m}")
            nc.sync.dma_start_transpose(t[:], xsl16_d[:, m * 128:(m + 1) * 128])
            xts.append(t)

        # ---- B/C projections -> bmat/cmat [N, SEQ] bf16 ----
        bmat = persist.tile([N, SEQ], B16, tag="bmat", name="bmat")
        cmat = persist.tile([N, SEQ], B16, tag="cmat", name="cmat")
        for tb in range(TB):
            psb = ps_mm.tile([128, 1024], F32, tag="mm", name="mmpb")
            for k in range(NK):
                nc.tensor.matmul(
                    psb[0:N, 0:512], wb_sb[k], xt[k][:, tb * 512:(tb + 1) * 512],
                    start=(k == 0), stop=(k == NK - 1),
                )
            for k in range(NK):
                nc.tensor.matmul(
                    psb[0:N, 512:1024], wc_sb[k],
                    xt[k][:, tb * 512:(tb + 1) * 512],
                    start=(k == 0), stop=(k == NK - 1),
                )
            nc.scalar.activation(
                bmat[:, tb * 512:(tb + 1) * 512], psb[0:N, 0:512],
                mybir.ActivationFunctionType.Identity, bias=bb_sb, scale=1.0,
            )
            nc.scalar.activation(
                cmat[:, tb * 512:(tb + 1) * 512], psb[0:N, 512:1024],
                mybir.ActivationFunctionType.Identity, bias=bc_sb, scale=1.0,
            )

        # bounce B/C to DRAM for partition-broadcast reads
        bmat_dr = dram.tile([N, SEQ], B16, tag="bmat_dr", name="bmat_dr")
        cmat_dr = dram.tile([N, SEQ], B16, tag="cmat_dr", name="cmat_dr")
        nc.sync.dma_start(bmat_dr[:], bmat[:])
        nc.sync.dma_start(cmat_dr[:], cmat[:])

        # ---- delta projection + softplus (Exp in-place on PSUM) ----
        dt16 = [persist.tile([128, SEQ], B16, tag=f"dt{m}", name=f"dtv{m}")
                for m in range(ND)]
        for m in range(ND):
            pss = []
            for th in range(2):
                ps = ps_mm.tile([128, 1024], F32, tag="mm", name="mmps")
                for sb in range(2):
                    for k in range(NK):
                        nc.tensor.matmul(
                            ps[:, sb * 512:(sb + 1) * 512],
                            wd_sb[k][:, m * 128:(m + 1) * 128],
                            xt[k][:, th * 1024 + sb * 512:th * 1024 + (sb + 1) * 512],
                            start=(k == 0), stop=(k == NK - 1),
                        )
                nc.scalar.activation(
                    ps[:], ps[:], mybir.ActivationFunctionType.Exp,
                    bias=bd_sb[m], scale=1.0,
                )
                pss.append(ps)
            for th in range(2):
                nc.scalar.activation(
                    dt16[m][:, th * 1024:(th + 1) * 1024], pss[th][:],
                    mybir.ActivationFunctionType.Ln, bias=1.0, scale=1.0,
                )

        # ---- dtx[m] = dt16[m]*x^T_own;  dskx[m] = x^T_own*D_skip (DVE) ----
        dtx = [persist.tile([128, SEQ], B16, tag=f"dtx{m}", name=f"dtx{m}")
               for m in range(ND)]
        dskx = [persist.tile([128, SEQ], B16, tag=f"dskx{m}", name=f"dskx{m}")
                for m in range(ND)]
        for m in range(ND):
            nc.vector.tensor_mul(dtx[m][:], dt16[m][:], xts[m][:])
            nc.vector.tensor_scalar_mul(dskx[m][:], xts[m][:], dsk_sb[m])

        # ---- scan phase ----
        y16 = [persist.tile([128, SEQ], B16, tag=f"y16{m}", name=f"y16{m}")
               for m in range(ND)]
        for half in range(NHALF):
            breps = []
            creps = []
            for j in range(NH):
                n = half * NH + j
                br = big16.tile([128, SEQ], B16, tag="big16", name="brep")
                nc.sync.dma_start(br[:], bmat_dr[n:n + 1, :].partition_broadcast(128))
                breps.append(br)
                cr = big16.tile([128, SEQ], B16, tag="big16", name="crep")
                nc.sync.dma_start(cr[:], cmat_dr[n:n + 1, :].partition_broadcast(128))
                creps.append(cr)
            for m in range(ND):
                yps = ps_y.tile([128, SEQ], F32, tag="yps", name="ypsv")
                # opening plane: skip term (half 0) / running y16 (half 1)
                opener = dskx[m] if half == 0 else y16[m]
                for tb in range(TB):
                    nc.tensor.matmul(
                        yps[:, tb * 512:(tb + 1) * 512], id16_sb[:],
                        opener[:, tb * 512:(tb + 1) * 512],
                        start=True, stop=False,
                    )
                for j in range(NH):
                    n = half * NH + j
                    bar = work.tile([128, SEQ], B16, tag="bar", name="barv",
                                    bufs=3)
                    nc.scalar.activation(
                        bar[:], dt16[m][:],
                        mybir.ActivationFunctionType.Exp,
                        bias=0.0, scale=a_sb[m][:, n:n + 1],
                    )
                    u = work.tile([128, SEQ], B16, tag="u", name="uv", bufs=3)
                    nc.vector.tensor_mul(u[:], dtx[m][:], breps[j][:])
                    h = work.tile([128, SEQ], B16, tag="h", name="hv")
                    nc.vector.tensor_tensor_scan(
                        h[:], bar[:], u[:], 0.0, op0=mult, op1=add,
                    )
                    prod = work.tile([128, SEQ], B16, tag="prod", name="prodv",
                                     bufs=3)
                    nc.vector.tensor_mul(prod[:], h[:], creps[j][:])
                    last = (j == NH - 1)
                    for tb in range(TB):
                        nc.tensor.matmul(
                            yps[:, tb * 512:(tb + 1) * 512], id16_sb[:],
                            prod[:, tb * 512:(tb + 1) * 512],
                            start=False, stop=last,
                        )
                nc.scalar.activation(
                    y16[m][:], yps[:],
                    mybir.ActivationFunctionType.Copy, bias=0.0, scale=1.0,
                )

        # ---- y^T via SBUF->SBUF xbar transpose, store bf16 ----
        for m in range(ND):
            ytt = work.tile([128, SEQ // 128, 128], B16, tag="ytt", name="yttv")
            nc.sync.dma_start_transpose(ytt[:], y16[m][:])
            nc.sync.dma_start(
                y_d[:, m * 128:(m + 1) * 128]
                .rearrange("(j p) q -> p j q", p=128),
                ytt[:],
            )

    nc.compile()
    _CACHE["nc"] = nc
    return nc


def _in_maps(x, A_log, D_skip, Wd, bd, Wb, bb, Wc, bc):
    A = (-np.exp(np.asarray(A_log, np.float64))).astype(np.float32)
    x = np.asarray(x, np.float32)
    maps = []
    for c in range(8):
        b, dh = c // 2, c % 2
        dsl = slice(dh * DL, (dh + 1) * DL)
        bdsk = np.stack([np.asarray(bd, np.float32)[dsl],
                         np.asarray(D_skip, np.float32)[dsl]], axis=1)
        bbcm = np.stack([np.asarray(bb, np.float32),
                         np.asarray(bc, np.float32)], axis=1)
        maps.append({
            "xb16": x[b].astype(BF16),
            "xsl16": x[b][:, dsl].astype(BF16),
            "wd16": np.asarray(Wd)[:, dsl].astype(BF16),
            "wb16": np.asarray(Wb).astype(BF16),
            "wc16": np.asarray(Wc).astype(BF16),
            "aneg": A[dsl],
            "bdsk": np.ascontiguousarray(bdsk),
            "bbc": np.ascontiguousarray(bbcm),
            "id16": np.eye(128, dtype=BF16),
        })
    return maps


def kernel(x, A_log, D_skip, Wd, bd, Wb, bb, Wc, bc, _trace=False):
    nc = _build()
    maps = _in_maps(x, A_log, D_skip, Wd, bd, Wb, bb, Wc, bc)
    res = run_bass_kernel_spmd(nc, maps, list(range(8)), trace=_trace)
    y = np.zeros((B_SZ, SEQ, D), np.float32)
    for c, om in enumerate(res.results):
        b, dh = c // 2, c % 2
        y[b][:, dh * DL:(dh + 1) * DL] = om["y"].astype(np.float32)
    if _trace:
        kernel.last_result = res
    return y
